# revision 37
# baseline (speedup 1.0000x reference)
"""DGCNN-style edge-conv block (KNN graph + dense conv stack) on 8 trn2 cores.

Strategy (data-parallel over batch, one batch element per core):
  scores   = <x_i,x_j> - 0.5*xx_j via one f32r PE matmul with [x; -0.5] x
             [x; x^2] contraction (the per-row -0.5*xx_i term is constant
             within a row, so it cannot change the row's top-k; dropped).
  top-16   = per-256-col-eighth DVE max8 (8 passes over 1/8 width) -> 64
             candidates; merge max8/match_replace/max8 on the 64-wide
             candidate tile; two full-width max_index passes recover the
             global indices. Exact unless one eighth holds >8 of a row's
             true top-16 (5 of 16384 rows for this input; error ~1e-3).
  gather   = P^T table (P = W1a @ x, 64 ch f32 = 256B rows) in DRAM,
             gathered in 1024-edge half-blocks with gpsimd dma_gather so
             each q-half's transposes start as soon as its half lands.
  edge MLP = A = relu(P_j + T_n), B2 = relu(W2a A + R_n),
             C3 = W3a A + W3c B2 + S_n, with T/R/S = per-node tables from
             small matmuls; per-edge convs run as bf16 block-diag matmuls
             on PE with 2k-stacked PE transposes. The R bias is folded into
             the conv2 PSUM group as an identity x R-broadcast bf16 matmul
             (plain-matmul accumulation is safe; mixing with is_transpose
             matmuls in one group crashes the device).
  output   = channel-concat [max_k A; x; max_k B2; max_k C3]; the k-max
             trees run on bf16 tiles (2x DVE rate), last op emits f32.

Schedule: all 16 row-tiles' scores+topk stream back-to-back on DVE (the
bottleneck engine); gather/MLP work is chunked 4,4,3,2,2,1 tiles so each
chunk's serial MLP latency hides under the remaining topk stream and only
a 1-tile chunk drains after the last top-k. The P/T/R/S table setup is
emitted first (for DRAM dep order) but at inflated scheduler priority so
the first score evacuations win the ACT queue.
"""

import numpy as np
import ml_dtypes

import bass_rust
import concourse.bass as bass
import concourse.bass_isa as bass_isa
import concourse.mybir as mybir
from concourse.bass_types import AP
from concourse.tile import TileContext
from concourse.bass_utils import run_bass_kernel_spmd
from concourse import dve_ops as _dve_ops
from concourse.dve_spec import Spec as _Spec, Src0 as _Src0, Src1 as _Src1, relu as _relu

# Fused A = relu(P^T + T) straight out of the transpose PSUM: one DVE pass
# replacing the scalar_tensor_tensor + ACT relu pair.
RELU_ADD_ANT = _dve_ops.DveOp(
    "RELU_ADD_ANT",
    _Spec(
        body=_relu(_Src0 + _Src1),
        reference=lambda in0, in1, s0, s1, imm2: np.maximum(
            in0.astype(np.float32) + in1, 0
        ).astype(np.float32),
    ),
    subdim=False,
    uops_sha={"v3": "1a5a0e13df7b5b5b", "v4": "762decb5a1a7b9ef"},
)
if RELU_ADD_ANT.name not in {o.name for o in _dve_ops.OPS}:
    _dve_ops.OPS.append(RELU_ADD_ANT)
    _dve_ops.CUSTOM_DVE_SPECS[RELU_ADD_ANT.name] = RELU_ADD_ANT.spec
    _dve_ops._SUB_OPCODE_FOR_NAME[RELU_ADD_ANT.name] = (
        _dve_ops._CUSTOM_DVE_ROW_BASE + len(_dve_ops.OPS) - 1
    )

F32 = mybir.dt.float32
F32R = mybir.dt.float32r
BF16 = mybir.dt.bfloat16
U16 = mybir.dt.uint16
I16 = mybir.dt.int16

B, C, N, K, G = 8, 64, 2048, 16, 64
NT = 16          # 128-row tiles
NE = 8           # candidate eighths per row for top-k
CHUNKS = [(0, 4), (4, 4), (8, 3), (11, 2), (13, 2), (15, 1)]  # (tile0, ntiles)
RELU = mybir.ActivationFunctionType.Relu
COPY = mybir.ActivationFunctionType.Copy
SQUARE = mybir.ActivationFunctionType.Square
ADD = mybir.AluOpType.add
MAX = mybir.AluOpType.max

_nop_ctr = [0]


def _split_all_waits(nc, max_waits=1):
    # This walrus build rejects >1 sync-wait on several CTRL structs; hoist
    # extras onto single-wait NOPs placed just before the instruction.
    for fn in nc.m.functions:
        for bb in fn.blocks:
            out = []
            for ins in bb.instructions:
                si = ins.sync_info
                if si is not None and si.on_wait is not None and len(si.on_wait) > max_waits:
                    waits = list(si.on_wait)
                    for w in waits[:-max_waits]:
                        _nop_ctr[0] += 1
                        nop = mybir.InstNoOp(name=f"waitnop-{_nop_ctr[0]}", ins=[], outs=[])
                        nop.engine = ins.engine
                        nop.sync_info = bass_rust.SyncInfo(on_wait=[w], on_update=[])
                        out.append(nop)
                        nc.register_instruction(nop, overwrite=True)
                    si.on_wait = waits[-max_waits:]
                out.append(ins)
            bb.instructions = out


def _insert_gpsimd_library_load(nc, lib_index=3):
    # InstDMAGatherAnt needs the 'mlp' GPSIMD ucode library; raw Bass+Tile
    # skips Bacc's insert_library_loads, so prepend the reload by hand.
    ins = bass_isa.InstPseudoReloadLibraryIndex(
        name="libload-manual", ins=[], outs=[], lib_index=lib_index
    )
    ins.engine = mybir.EngineType.Pool
    nc.register_instruction(ins, overwrite=True)
    bb0 = nc.m.functions[0].blocks[0]
    bb0.instructions = [ins] + list(bb0.instructions)
    mybir.codegen_inst_isa_subclasses(nc)


def build():
    nc = bass.Bass("TRN2", debug=False, num_devices=8)

    x_in = nc.dram_tensor("x", [C, N], F32, kind="ExternalInput")
    XR = nc.dram_tensor("XR", [C, N], F32R, kind="ExternalInput")
    NEGH = nc.dram_tensor("NEGH", [64, N], F32R, kind="ExternalInput")  # -0.5 const
    ONESR = nc.dram_tensor("ONESR", [1, N], F32R, kind="ExternalInput")
    WLTP = nc.dram_tensor("WLTP", [64, 64], F32R, kind="ExternalInput")   # W1a.T
    WLT = nc.dram_tensor("WLT", [65, 64], F32R, kind="ExternalInput")     # [(W1b-W1a).T; b1]
    WLR = nc.dram_tensor("WLR", [65, 64], F32R, kind="ExternalInput")     # [W2b.T; b2]
    WLS = nc.dram_tensor("WLS", [65, 64], F32R, kind="ExternalInput")     # [W3b.T; b3]
    W2BLK = nc.dram_tensor("W2BLK", [128, 128], BF16, kind="ExternalInput")
    W3ABLK = nc.dram_tensor("W3ABLK", [128, 128], BF16, kind="ExternalInput")
    W3CBLK = nc.dram_tensor("W3CBLK", [128, 128], BF16, kind="ExternalInput")
    EYE = nc.dram_tensor("EYE", [128, 128], F32, kind="ExternalInput")
    EYE16 = nc.dram_tensor("EYE16", [128, 128], BF16, kind="ExternalInput")
    Y = nc.dram_tensor("y", [C + 3 * G, N], F32, kind="ExternalOutput")

    PT_D = nc.dram_tensor("PT_D", [N, C], F32, kind="Internal")
    IDXD = nc.dram_tensor("IDXD", [N * K], I16, kind="Internal")

    with TileContext(nc) as tc:
        with tc.tile_pool(name="const", bufs=1) as cp, \
             tc.tile_pool(name="work", bufs=2) as wp, \
             tc.tile_pool(name="chunk", bufs=2) as kp, \
             tc.tile_pool(name="gat", bufs=2) as gp, \
             tc.tile_pool(name="ps2", bufs=2, space="PSUM") as pps2, \
             tc.tile_pool(name="ps1", bufs=2, space="PSUM") as pps1:

            # ---------------- setup ----------------
            X65 = cp.tile([65, N], F32R)
            RHSB = cp.tile([128, N], F32R)
            LHSB = cp.tile([128, N], F32R)
            PC = cp.tile([64, N], F32)
            TSTK = cp.tile([128, N], BF16)
            RSTK = cp.tile([128, N], BF16)
            SCt = cp.tile([64, N], F32)
            PTS = cp.tile([128, NT * 64], F32)
            IDXALL = cp.tile([128, NT * K], U16)
            EYEt = cp.tile([128, 128], F32)
            EYE16t = cp.tile([128, 128], BF16)
            wltp = cp.tile([64, 64], F32R)
            wlt = cp.tile([65, 64], F32R)
            wlr = cp.tile([65, 64], F32R)
            wls = cp.tile([65, 64], F32R)
            w2b = cp.tile([128, 128], BF16)
            w3a = cp.tile([128, 128], BF16)
            w3c = cp.tile([128, 128], BF16)

            nc.sync.dma_start(out=EYE16t[:, :], in_=EYE16[:, :])
            nc.sync.dma_start(out=RHSB[0:64, :], in_=XR[:, :])
            nc.sync.dma_start(out=LHSB[0:64, :], in_=XR[:, :])
            nc.sync.dma_start(out=LHSB[64:128, :], in_=NEGH[:, :])
            nc.sync.dma_start(out=X65[0:64, :], in_=XR[:, :])
            nc.sync.dma_start(out=X65[64:65, :], in_=ONESR[:, :])
            nc.sync.dma_start(out=EYEt[:, :], in_=EYE[:, :])
            nc.sync.dma_start(out=wltp[:, :], in_=WLTP[:, :])
            nc.sync.dma_start(out=wlt[:, :], in_=WLT[:, :])
            nc.sync.dma_start(out=wlr[:, :], in_=WLR[:, :])
            nc.sync.dma_start(out=wls[:, :], in_=WLS[:, :])
            # RHSB lower half = x^2 on the (idle-at-head) DVE, in halves so
            # the first score matmul starts after half the square.
            nc.vector.tensor_tensor(out=RHSB[64:128, 0:1024],
                                    in0=RHSB[0:64, 0:1024].bitcast(F32),
                                    in1=RHSB[0:64, 0:1024].bitcast(F32),
                                    op=mybir.AluOpType.mult)
            nc.vector.tensor_tensor(out=RHSB[64:128, 1024:2048],
                                    in0=RHSB[0:64, 1024:2048].bitcast(F32),
                                    in1=RHSB[0:64, 1024:2048].bitcast(F32),
                                    op=mybir.AluOpType.mult)
            ps_warm = pps1.tile([128, 1], F32, tag="u2")
            nc.tensor.matmul(ps_warm[:, :], EYE16t[:, :], EYE16t[:, 0:1],
                             start=True, stop=True)
            nc.sync.dma_start(out=w2b[:, :], in_=W2BLK[:, :])
            nc.sync.dma_start(out=w3a[:, :], in_=W3ABLK[:, :])
            nc.sync.dma_start(out=w3c[:, :], in_=W3CBLK[:, :])

            # ---------------- table setup ----------------
            # Emitted before the topk loop so DRAM deps (PT_D) order
            # correctly, but with inflated scheduler priority so the score
            # pipeline wins the engine queues at the head.
            _p0 = tc.cur_priority
            tc.cur_priority = _p0 + 100000

            # P (c-layout), T/R stacked, S  — small f32r matmuls
            for u in range(4):
                sl = slice(u * 512, (u + 1) * 512)
                p1 = pps1.tile([64, 512], F32, tag="u2")
                nc.tensor.matmul(p1[:, :], wltp[:, :], RHSB[0:64, sl], start=True, stop=True)
                nc.scalar.activation(PC[:, sl], p1[:, :], COPY)
                p2 = pps1.tile([64, 512], F32, tag="u2")
                nc.tensor.matmul(p2[:, :], wlt[:, :], X65[:, sl], start=True, stop=True)
                nc.scalar.activation(TSTK[0:64, sl], p2[:, :], COPY)
                p3 = pps1.tile([64, 512], F32, tag="u2")
                nc.tensor.matmul(p3[:, :], wlr[:, :], X65[:, sl], start=True, stop=True)
                nc.scalar.activation(RSTK[0:64, sl], p3[:, :], COPY)
                p4 = pps1.tile([64, 512], F32, tag="u2")
                nc.tensor.matmul(p4[:, :], wls[:, :], X65[:, sl], start=True, stop=True)
                nc.scalar.activation(SCt[:, sl], p4[:, :], COPY)
            nc.sync.dma_start(out=TSTK[64:128, :], in_=TSTK[0:64, :])
            nc.sync.dma_start(out=RSTK[64:128, :], in_=RSTK[0:64, :])

            # P^T table -> DRAM (256B rows)
            for rt in range(NT):
                pt = pps1.tile([128, 64], F32, tag="u2")
                nc.tensor.transpose(pt[:, :], PC[:, rt * 128:(rt + 1) * 128],
                                    EYEt[0:64, 0:64])
                nc.scalar.activation(PTS[:, rt * 64:(rt + 1) * 64], pt[:, :], COPY)
            nc.sync.dma_start(
                out=AP(PT_D, 0, [[64, 128], [8192, NT], [1, 64]]),
                in_=PTS[:, :].rearrange("p (a b) -> p a b", a=NT),
            )

            # x passthrough output rows 64:128
            nc.sync.dma_start(out=Y[64:128, :], in_=x_in[:, :])

            tc.cur_priority = _p0

            # ---------------- scores + topk for all row tiles ----------------
            chunk_of = {}
            for ci, (t0, nb) in enumerate(CHUNKS):
                for t in range(t0, t0 + nb):
                    chunk_of[t] = (ci, t0, nb)

            for rt in range(NT):
                SCORES = wp.tile([128, N], F32, tag="scores")
                for u in range(2):
                    pss = pps2.tile([128, 1024], F32, tag="score")
                    for h in range(2):
                        nc.tensor.matmul(pss[:, h * 512:(h + 1) * 512],
                                         LHSB[:, rt * 128:(rt + 1) * 128],
                                         RHSB[:, u * 1024 + h * 512:u * 1024 + (h + 1) * 512],
                                         start=True, stop=True)
                    nc.scalar.activation(SCORES[:, u * 1024:(u + 1) * 1024],
                                         pss[:, :], COPY)
                # per-eighth candidate max8s (each 256-wide)
                C64 = wp.tile([128, NE * 8], F32, tag="c64")
                for e in range(NE):
                    nc.vector.max(out=C64[:, e * 8:(e + 1) * 8],
                                  in_=SCORES[:, e * 256:(e + 1) * 256])
                f8a = wp.tile([128, 8], F32, tag="f8a")
                f8b = wp.tile([128, 8], F32, tag="f8b")
                nc.vector.max(out=f8a[:, :], in_=C64[:, :])
                nc.vector.match_replace(out=C64[:, :], in_to_replace=f8a[:, :],
                                        in_values=C64[:, :], imm_value=-3.0e38)
                nc.vector.max(out=f8b[:, :], in_=C64[:, :])
                nc.vector.max_index(out=IDXALL[:, rt * K:rt * K + 8],
                                    in_max=f8a[:, :], in_values=SCORES[:, :])
                nc.vector.max_index(out=IDXALL[:, rt * K + 8:rt * K + 16],
                                    in_max=f8b[:, :], in_values=SCORES[:, :])

            for ci, (t0, nb) in enumerate(CHUNKS):
                _emit_chunk(nc, tc, ci, t0, nb,
                            IDXD, IDXALL, PT_D, TSTK, RSTK, SCt,
                            EYEt, EYE16t, w2b, w3a, w3c, Y,
                            gp, kp, wp, pps1, pps2)

    _split_all_waits(nc)
    _insert_gpsimd_library_load(nc, 3)
    return nc


def _emit_chunk(nc, tc, ci, t0, nb,
                IDXD, IDXALL, PT_D, TSTK, RSTK, SCt,
                EYEt, EYE16t, w2b, w3a, w3c, Y,
                gp, kp, wp, pps1, pps2):
    """Gather + edge MLP + k-max trees for a chunk of `nb` row tiles
    (nodes t0*128 .. (t0+nb)*128)."""
    W = nb * 16          # idx row stride within this chunk's IDXD block
    # idx -> DRAM (addr = t0*2048 + r*W + bl*16 + k) -> wrapped read.
    nc.sync.dma_start(
        out=AP(IDXD, t0 * 2048, [[W, 128], [16, nb], [1, K]]),
        in_=IDXALL[:, t0 * 16:(t0 + nb) * 16].bitcast(I16)
            .rearrange("p (a b) -> p a b", a=nb),
    )
    idxt = gp.tile([128, nb * 128], I16, tag="idxt")
    # idxt[p, s], s = bl*128 + k*8 + rhi  <-  addr p*W + rhi*(16*W) + bl*16 + k
    src_w = AP(IDXD, t0 * 2048, [[W, 16], [16, nb], [1, K], [16 * W, 8]])
    nc.sync.dma_start(out=idxt[0:16, :], in_=src_w)
    nc.sync.dma_start(out=idxt[16:32, :], in_=idxt[0:16, :])
    nc.sync.dma_start(out=idxt[32:64, :], in_=idxt[0:32, :])
    nc.sync.dma_start(out=idxt[64:128, :], in_=idxt[0:64, :])

    PG = gp.tile([128, nb * 16, 64], F32, tag="pg")
    for bl in range(nb):
        for qh in range(2):
            nc.gpsimd.dma_gather(
                out_ap=PG[:, bl * 16 + qh * 8:bl * 16 + qh * 8 + 8, :],
                in_ap=PT_D.ap(),
                idxs_ap=idxt[:, bl * 128 + qh * 64:bl * 128 + qh * 64 + 64],
                num_idxs=1024, num_idxs_reg=1024, elem_size=64,
                single_packet=False,
            )

    AC = kp.tile([128, nb, 8, 128], BF16, tag="ac")
    B2C = kp.tile([128, nb, 8, 128], BF16, tag="b2c")
    C3C = kp.tile([128, nb, 8, 128], BF16, tag="c3c")

    for bl in range(nb):
        g = t0 + bl
        tb = TSTK[:, g * 128:(g + 1) * 128].unsqueeze(1).broadcast_to([128, 4, 128])
        rb = RSTK[:, g * 128:(g + 1) * 128].unsqueeze(1).broadcast_to([128, 4, 128])
        for q in range(2):
            # transposes: 4 kp blocks -> psum (128, 512)
            psa = pps2.tile([128, 512], F32, tag="a")
            for kk in range(4):
                kpi = q * 4 + kk
                blk = PG[:, bl * 16 + 2 * kpi:bl * 16 + 2 * kpi + 2, :]
                nc.tensor.transpose(psa[:, kk * 128:(kk + 1) * 128],
                                    blk, EYEt[:, :])
            nc.vector._custom_dve(
                RELU_ADD_ANT,
                out=AC[:, bl, 4 * q:4 * q + 4, :],
                in0=psa[:, :].rearrange("p (a b) -> p a b", a=4),
                in1=tb,
            )

            # conv2 (+R folded in as an identity-matmul accumulate)
            ps2t = pps1.tile([128, 512], F32, tag="u2")
            nc.tensor.matmul(ps2t[:, :], w2b[:, :],
                             AC[:, bl, 4 * q:4 * q + 4, :],
                             start=True, stop=False,
                             skip_group_check=True)
            nc.tensor.matmul(ps2t[:, :].rearrange("p (a b) -> p a b", a=4),
                             EYE16t[:, :], rb,
                             start=False, stop=True,
                             skip_group_check=True)
            nc.scalar.activation(B2C[:, bl, 4 * q:4 * q + 4, :], ps2t[:, :], RELU)

            # conv3 (accumulate two matmuls)
            ps3t = pps1.tile([128, 512], F32, tag="u2")
            nc.tensor.matmul(ps3t[:, :], w3a[:, :],
                             AC[:, bl, 4 * q:4 * q + 4, :],
                             start=True, stop=False)
            nc.tensor.matmul(ps3t[:, :], w3c[:, :],
                             B2C[:, bl, 4 * q:4 * q + 4, :],
                             start=False, stop=True)
            nc.scalar.activation(C3C[:, bl, 4 * q:4 * q + 4, :], ps3t[:, :], COPY)

    # maxes over k (kp in free dim, then k-parity across the 64-partition
    # halves), then DMA the chunk's output columns. Tree runs in bf16
    # (2x DVE); the last op per source emits f32.
    for (src, row0, add_s) in ((AC, 0, False), (B2C, 2 * G, False), (C3C, 3 * G, True)):
        m1 = kp.tile([128, nb, 4, 128], BF16, tag="m1")
        nc.vector.tensor_tensor(out=m1[:, :, :, :], in0=src[:, :, 0:4, :],
                                in1=src[:, :, 4:8, :], op=MAX)
        m2 = kp.tile([128, nb, 2, 128], BF16, tag="m2")
        nc.vector.tensor_tensor(out=m2[:, :, :, :], in0=m1[:, :, 0:2, :],
                                in1=m1[:, :, 2:4, :], op=MAX)
        red = kp.tile([128, nb, 128], BF16, tag=f"red{row0}")
        nc.vector.tensor_tensor(out=red[:, :, :], in0=m2[:, :, 0, :],
                                in1=m2[:, :, 1, :], op=MAX)
        hi = kp.tile([64, nb * 128], BF16, tag=f"hi{row0}")
        nc.scalar.activation(hi[:, :],
                             red[64:128, :, :].rearrange("p a n -> p (a n)"),
                             COPY)
        om = kp.tile([64, nb * 128], F32, tag=f"om{row0}")
        if add_s:
            omh = kp.tile([64, nb * 128], BF16, tag="omh")
            nc.vector.tensor_tensor(out=omh[:, :],
                                    in0=red[0:64, :, :].rearrange("p a n -> p (a n)"),
                                    in1=hi[:, :], op=MAX)
            nc.vector.tensor_tensor(out=om[:, :], in0=omh[:, :],
                                    in1=SCt[:, t0 * 128:(t0 + nb) * 128],
                                    op=ADD)
        else:
            omb = kp.tile([64, nb * 128], BF16, tag=f"omb{row0}")
            nc.vector.tensor_tensor(out=omb[:, :],
                                    in0=red[0:64, :, :].rearrange("p a n -> p (a n)"),
                                    in1=hi[:, :], op=MAX)
            nc.scalar.activation(om[:, :], omb[:, :], COPY)
        nc.sync.dma_start(out=Y[row0:row0 + 64, t0 * 128:(t0 + nb) * 128],
                          in_=om[:, :])


def _prep_weights(W1, b1, W2, b2, W3, b3):
    W1 = np.asarray(W1, np.float32); W2 = np.asarray(W2, np.float32)
    W3 = np.asarray(W3, np.float32)
    b1 = np.asarray(b1, np.float32); b2 = np.asarray(b2, np.float32)
    b3 = np.asarray(b3, np.float32)
    W1a, W1b = W1[:, :64], W1[:, 64:]
    W2a, W2b = W2[:, :64], W2[:, 64:]
    W3a, W3b, W3c = W3[:, :64], W3[:, 64:128], W3[:, 128:]

    def blk(w):
        z = np.zeros((128, 128), np.float32)
        z[0:64, 0:64] = w.T
        z[64:128, 64:128] = w.T
        return z.astype(ml_dtypes.bfloat16)

    return {
        "NEGH": np.full((64, N), -0.5, np.float32),
        "ONESR": np.ones((1, N), np.float32),
        "WLTP": np.ascontiguousarray(W1a.T),
        "WLT": np.ascontiguousarray(np.vstack([(W1b - W1a).T, b1[None, :]])),
        "WLR": np.ascontiguousarray(np.vstack([W2b.T, b2[None, :]])),
        "WLS": np.ascontiguousarray(np.vstack([W3b.T, b3[None, :]])),
        "W2BLK": blk(W2a),
        "W3ABLK": blk(W3a),
        "W3CBLK": blk(W3c),
        "EYE": np.eye(128, dtype=np.float32),
        "EYE16": np.eye(128, dtype=np.float32).astype(ml_dtypes.bfloat16),
    }


_NC = None


def kernel(x, W1, b1, W2, b2, W3, b3):
    global _NC
    if _NC is None:
        _NC = build()
    x = np.asarray(x, np.float32)
    w = _prep_weights(W1, b1, W2, b2, W3, b3)
    in_maps = [{"x": np.ascontiguousarray(x[b]), "XR": np.ascontiguousarray(x[b]), **w}
               for b in range(B)]
    res = run_bass_kernel_spmd(_NC, in_maps, core_ids=list(range(B)))
    return np.stack([res.results[b]["y"] for b in range(B)], axis=0)


# revision 40
# speedup vs baseline: 1.0062x; 1.0062x over previous
"""DGCNN-style edge-conv block (KNN graph + dense conv stack) on 8 trn2 cores.

Strategy (data-parallel over batch, one batch element per core):
  scores   = <x_i,x_j> - 0.5*xx_j via one f32r PE matmul with [x; -0.5] x
             [x; x^2] contraction (the per-row -0.5*xx_i term is constant
             within a row, so it cannot change the row's top-k; dropped).
  top-16   = per-256-col-eighth DVE max8 (8 passes over 1/8 width) -> 64
             candidates; merge max8/match_replace/max8 on the 64-wide
             candidate tile; two full-width max_index passes recover the
             global indices. Exact unless one eighth holds >8 of a row's
             true top-16 (5 of 16384 rows for this input; error ~1e-3).
  gather   = P^T table (P = W1a @ x, 64 ch f32 = 256B rows) in DRAM,
             gathered in 1024-edge half-blocks with gpsimd dma_gather so
             each q-half's transposes start as soon as its half lands.
  edge MLP = A = relu(P_j + T_n), B2 = relu(W2a A + R_n),
             C3 = W3a A + W3c B2 + S_n, with T/R/S = per-node tables from
             small matmuls; per-edge convs run as bf16 block-diag matmuls
             on PE with 2k-stacked PE transposes. The R bias is folded into
             the conv2 PSUM group as an identity x R-broadcast bf16 matmul
             (plain-matmul accumulation is safe; mixing with is_transpose
             matmuls in one group crashes the device).
  output   = channel-concat [max_k A; x; max_k B2; max_k C3]; the k-max
             trees run on bf16 tiles (2x DVE rate), last op emits f32.

Schedule: all 16 row-tiles' scores+topk stream back-to-back on DVE (the
bottleneck engine); gather/MLP work is chunked 4,4,3,2,2,1 tiles so each
chunk's serial MLP latency hides under the remaining topk stream and only
a 1-tile chunk drains after the last top-k. The P/T/R/S table setup is
emitted first (for DRAM dep order) but at inflated scheduler priority so
the first score evacuations win the ACT queue.
"""

import numpy as np
import ml_dtypes

import bass_rust
import concourse.bass as bass
import concourse.bass_isa as bass_isa
import concourse.mybir as mybir
from concourse.bass_types import AP
from concourse.tile import TileContext
from concourse.bass_utils import run_bass_kernel_spmd
from concourse import dve_ops as _dve_ops
from concourse.dve_spec import Spec as _Spec, Src0 as _Src0, Src1 as _Src1, relu as _relu

# Fused A = relu(P^T + T) straight out of the transpose PSUM: one DVE pass
# replacing the scalar_tensor_tensor + ACT relu pair.
RELU_ADD_ANT = _dve_ops.DveOp(
    "RELU_ADD_ANT",
    _Spec(
        body=_relu(_Src0 + _Src1),
        reference=lambda in0, in1, s0, s1, imm2: np.maximum(
            in0.astype(np.float32) + in1, 0
        ).astype(np.float32),
    ),
    subdim=False,
    uops_sha={"v3": "1a5a0e13df7b5b5b", "v4": "762decb5a1a7b9ef"},
)
if RELU_ADD_ANT.name not in {o.name for o in _dve_ops.OPS}:
    _dve_ops.OPS.append(RELU_ADD_ANT)
    _dve_ops.CUSTOM_DVE_SPECS[RELU_ADD_ANT.name] = RELU_ADD_ANT.spec
    _dve_ops._SUB_OPCODE_FOR_NAME[RELU_ADD_ANT.name] = (
        _dve_ops._CUSTOM_DVE_ROW_BASE + len(_dve_ops.OPS) - 1
    )

F32 = mybir.dt.float32
F32R = mybir.dt.float32r
BF16 = mybir.dt.bfloat16
U16 = mybir.dt.uint16
I16 = mybir.dt.int16

B, C, N, K, G = 8, 64, 2048, 16, 64
NT = 16          # 128-row tiles
NE = 8           # candidate eighths per row for top-k
CHUNKS = [(0, 4), (4, 4), (8, 3), (11, 2), (13, 2), (15, 1)]  # (tile0, ntiles)
RELU = mybir.ActivationFunctionType.Relu
COPY = mybir.ActivationFunctionType.Copy
SQUARE = mybir.ActivationFunctionType.Square
ADD = mybir.AluOpType.add
MAX = mybir.AluOpType.max

_nop_ctr = [0]


def _split_all_waits(nc, max_waits=1):
    # This walrus build rejects >1 sync-wait on several CTRL structs; hoist
    # extras onto single-wait NOPs placed just before the instruction.
    for fn in nc.m.functions:
        for bb in fn.blocks:
            out = []
            for ins in bb.instructions:
                si = ins.sync_info
                if si is not None and si.on_wait is not None and len(si.on_wait) > max_waits:
                    waits = list(si.on_wait)
                    for w in waits[:-max_waits]:
                        _nop_ctr[0] += 1
                        nop = mybir.InstNoOp(name=f"waitnop-{_nop_ctr[0]}", ins=[], outs=[])
                        nop.engine = ins.engine
                        nop.sync_info = bass_rust.SyncInfo(on_wait=[w], on_update=[])
                        out.append(nop)
                        nc.register_instruction(nop, overwrite=True)
                    si.on_wait = waits[-max_waits:]
                out.append(ins)
            bb.instructions = out


def _insert_gpsimd_library_load(nc, lib_index=3):
    # InstDMAGatherAnt needs the 'mlp' GPSIMD ucode library; raw Bass+Tile
    # skips Bacc's insert_library_loads, so prepend the reload by hand.
    ins = bass_isa.InstPseudoReloadLibraryIndex(
        name="libload-manual", ins=[], outs=[], lib_index=lib_index
    )
    ins.engine = mybir.EngineType.Pool
    nc.register_instruction(ins, overwrite=True)
    bb0 = nc.m.functions[0].blocks[0]
    bb0.instructions = [ins] + list(bb0.instructions)
    mybir.codegen_inst_isa_subclasses(nc)


def build():
    nc = bass.Bass("TRN2", debug=False, num_devices=8)

    x_in = nc.dram_tensor("x", [C, N], F32, kind="ExternalInput")
    XR = nc.dram_tensor("XR", [C, N], F32R, kind="ExternalInput")
    NEGH = nc.dram_tensor("NEGH", [64, N], F32R, kind="ExternalInput")  # -0.5 const
    ONESR = nc.dram_tensor("ONESR", [1, N], F32R, kind="ExternalInput")
    WLTP = nc.dram_tensor("WLTP", [64, 64], F32R, kind="ExternalInput")   # W1a.T
    WLT = nc.dram_tensor("WLT", [65, 64], F32R, kind="ExternalInput")     # [(W1b-W1a).T; b1]
    WLR = nc.dram_tensor("WLR", [65, 64], F32R, kind="ExternalInput")     # [W2b.T; b2]
    WLS = nc.dram_tensor("WLS", [65, 64], F32R, kind="ExternalInput")     # [W3b.T; b3]
    W2BLK = nc.dram_tensor("W2BLK", [128, 128], BF16, kind="ExternalInput")
    W3ABLK = nc.dram_tensor("W3ABLK", [128, 128], BF16, kind="ExternalInput")
    W3CBLK = nc.dram_tensor("W3CBLK", [128, 128], BF16, kind="ExternalInput")
    EYE = nc.dram_tensor("EYE", [128, 128], F32, kind="ExternalInput")
    EYE16 = nc.dram_tensor("EYE16", [128, 128], BF16, kind="ExternalInput")
    Y = nc.dram_tensor("y", [C + 3 * G, N], F32, kind="ExternalOutput")

    PT_D = nc.dram_tensor("PT_D", [N, C], F32, kind="Internal")
    IDXD = nc.dram_tensor("IDXD", [N * K], I16, kind="Internal")

    with TileContext(nc) as tc:
        with tc.tile_pool(name="const", bufs=1) as cp, \
             tc.tile_pool(name="work", bufs=2) as wp, \
             tc.tile_pool(name="chunk", bufs=2) as kp, \
             tc.tile_pool(name="gat", bufs=2) as gp, \
             tc.tile_pool(name="ps2", bufs=2, space="PSUM") as pps2, \
             tc.tile_pool(name="ps1", bufs=2, space="PSUM") as pps1:

            # ---------------- setup ----------------
            X65 = cp.tile([65, N], F32R)
            RHSB = cp.tile([128, N], F32R)
            LHSB = cp.tile([128, N], F32R)
            PC = cp.tile([64, N], F32)
            TSTK = cp.tile([128, N], BF16)
            RSTK = cp.tile([128, N], BF16)
            SCt = cp.tile([64, N], F32)
            PTS = cp.tile([128, NT * 64], F32)
            IDXALL = cp.tile([128, NT * K], U16)
            EYEt = cp.tile([128, 128], F32)
            EYE16t = cp.tile([128, 128], BF16)
            wltp = cp.tile([64, 64], F32R)
            wlt = cp.tile([65, 64], F32R)
            wlr = cp.tile([65, 64], F32R)
            wls = cp.tile([65, 64], F32R)
            w2b = cp.tile([128, 128], BF16)
            w3a = cp.tile([128, 128], BF16)
            w3c = cp.tile([128, 128], BF16)

            nc.sync.dma_start(out=EYE16t[:, :], in_=EYE16[:, :])
            nc.sync.dma_start(out=RHSB[0:64, :], in_=XR[:, :])
            nc.sync.dma_start(out=LHSB[0:64, :], in_=XR[:, :])
            nc.sync.dma_start(out=LHSB[64:128, :], in_=NEGH[:, :])
            nc.sync.dma_start(out=X65[0:64, :], in_=XR[:, :])
            nc.sync.dma_start(out=X65[64:65, :], in_=ONESR[:, :])
            nc.sync.dma_start(out=EYEt[:, :], in_=EYE[:, :])
            nc.sync.dma_start(out=wltp[:, :], in_=WLTP[:, :])
            nc.sync.dma_start(out=wlt[:, :], in_=WLT[:, :])
            nc.sync.dma_start(out=wlr[:, :], in_=WLR[:, :])
            nc.sync.dma_start(out=wls[:, :], in_=WLS[:, :])
            # RHSB lower half = x^2 on the (idle-at-head) DVE, in halves so
            # the first score matmul starts after half the square.
            nc.vector.tensor_tensor(out=RHSB[64:128, 0:1024],
                                    in0=RHSB[0:64, 0:1024].bitcast(F32),
                                    in1=RHSB[0:64, 0:1024].bitcast(F32),
                                    op=mybir.AluOpType.mult)
            nc.vector.tensor_tensor(out=RHSB[64:128, 1024:2048],
                                    in0=RHSB[0:64, 1024:2048].bitcast(F32),
                                    in1=RHSB[0:64, 1024:2048].bitcast(F32),
                                    op=mybir.AluOpType.mult)
            ps_warm = pps1.tile([128, 1], F32, tag="u2")
            nc.tensor.matmul(ps_warm[:, :], EYE16t[:, :], EYE16t[:, 0:1],
                             start=True, stop=True)
            nc.sync.dma_start(out=w2b[:, :], in_=W2BLK[:, :])
            nc.sync.dma_start(out=w3a[:, :], in_=W3ABLK[:, :])
            nc.sync.dma_start(out=w3c[:, :], in_=W3CBLK[:, :])

            # ---------------- table setup ----------------
            # Emitted before the topk loop so DRAM deps (PT_D) order
            # correctly, but with inflated scheduler priority so the score
            # pipeline wins the engine queues at the head.
            _p0 = tc.cur_priority
            tc.cur_priority = _p0 + 100000

            # P (c-layout), T/R stacked, S  — small f32r matmuls
            for u in range(4):
                sl = slice(u * 512, (u + 1) * 512)
                p1 = pps1.tile([64, 512], F32, tag="u2")
                nc.tensor.matmul(p1[:, :], wltp[:, :], RHSB[0:64, sl], start=True, stop=True)
                nc.scalar.activation(PC[:, sl], p1[:, :], COPY)
                p2 = pps1.tile([64, 512], F32, tag="u2")
                nc.tensor.matmul(p2[:, :], wlt[:, :], X65[:, sl], start=True, stop=True)
                nc.scalar.activation(TSTK[0:64, sl], p2[:, :], COPY)
                p3 = pps1.tile([64, 512], F32, tag="u2")
                nc.tensor.matmul(p3[:, :], wlr[:, :], X65[:, sl], start=True, stop=True)
                nc.scalar.activation(RSTK[0:64, sl], p3[:, :], COPY)
                p4 = pps1.tile([64, 512], F32, tag="u2")
                nc.tensor.matmul(p4[:, :], wls[:, :], X65[:, sl], start=True, stop=True)
                nc.scalar.activation(SCt[:, sl], p4[:, :], COPY)
            nc.sync.dma_start(out=TSTK[64:128, :], in_=TSTK[0:64, :])
            nc.sync.dma_start(out=RSTK[64:128, :], in_=RSTK[0:64, :])

            # P^T table -> DRAM (256B rows)
            for rt in range(NT):
                pt = pps1.tile([128, 64], F32, tag="u2")
                nc.tensor.transpose(pt[:, :], PC[:, rt * 128:(rt + 1) * 128],
                                    EYEt[0:64, 0:64])
                nc.scalar.activation(PTS[:, rt * 64:(rt + 1) * 64], pt[:, :], COPY)
            nc.sync.dma_start(
                out=AP(PT_D, 0, [[64, 128], [8192, NT], [1, 64]]),
                in_=PTS[:, :].rearrange("p (a b) -> p a b", a=NT),
            )

            # x passthrough output rows 64:128
            nc.sync.dma_start(out=Y[64:128, :], in_=x_in[:, :])

            tc.cur_priority = _p0

            # ---------------- scores + topk for all row tiles ----------------
            chunk_of = {}
            for ci, (t0, nb) in enumerate(CHUNKS):
                for t in range(t0, t0 + nb):
                    chunk_of[t] = (ci, t0, nb)

            for rt in range(NT):
                SCORES = wp.tile([128, N], F32, tag="scores")
                for u in range(2):
                    pss = pps2.tile([128, 1024], F32, tag="score")
                    for h in range(2):
                        nc.tensor.matmul(pss[:, h * 512:(h + 1) * 512],
                                         LHSB[:, rt * 128:(rt + 1) * 128],
                                         RHSB[:, u * 1024 + h * 512:u * 1024 + (h + 1) * 512],
                                         start=True, stop=True)
                    nc.scalar.activation(SCORES[:, u * 1024:(u + 1) * 1024],
                                         pss[:, :], COPY)
                # per-eighth candidate max8s (each 256-wide)
                C64 = wp.tile([128, NE * 8], F32, tag="c64")
                for e in range(NE):
                    nc.vector.max(out=C64[:, e * 8:(e + 1) * 8],
                                  in_=SCORES[:, e * 256:(e + 1) * 256])
                f8a = wp.tile([128, 8], F32, tag="f8a")
                f8b = wp.tile([128, 8], F32, tag="f8b")
                nc.vector.max(out=f8a[:, :], in_=C64[:, :])
                nc.vector.match_replace(out=C64[:, :], in_to_replace=f8a[:, :],
                                        in_values=C64[:, :], imm_value=-3.0e38)
                nc.vector.max(out=f8b[:, :], in_=C64[:, :])
                nc.vector.max_index(out=IDXALL[:, rt * K:rt * K + 8],
                                    in_max=f8a[:, :], in_values=SCORES[:, :])
                nc.vector.max_index(out=IDXALL[:, rt * K + 8:rt * K + 16],
                                    in_max=f8b[:, :], in_values=SCORES[:, :])

            for ci, (t0, nb) in enumerate(CHUNKS):
                _emit_chunk(nc, tc, ci, t0, nb,
                            IDXD, IDXALL, PT_D, TSTK, RSTK, SCt,
                            EYEt, EYE16t, w2b, w3a, w3c, Y,
                            gp, kp, wp, pps1, pps2)

    _split_all_waits(nc)
    _insert_gpsimd_library_load(nc, 3)
    return nc


def _emit_chunk(nc, tc, ci, t0, nb,
                IDXD, IDXALL, PT_D, TSTK, RSTK, SCt,
                EYEt, EYE16t, w2b, w3a, w3c, Y,
                gp, kp, wp, pps1, pps2):
    """Gather + edge MLP + k-max trees for a chunk of `nb` row tiles
    (nodes t0*128 .. (t0+nb)*128)."""
    W = nb * 16          # idx row stride within this chunk's IDXD block
    # idx -> DRAM (addr = t0*2048 + r*W + bl*16 + k) -> wrapped read.
    nc.sync.dma_start(
        out=AP(IDXD, t0 * 2048, [[W, 128], [16, nb], [1, K]]),
        in_=IDXALL[:, t0 * 16:(t0 + nb) * 16].bitcast(I16)
            .rearrange("p (a b) -> p a b", a=nb),
    )
    idxt = gp.tile([128, nb * 128], I16, tag="idxt")
    # idxt[p, s], s = bl*128 + k*8 + rhi  <-  addr p*W + rhi*(16*W) + bl*16 + k
    src_w = AP(IDXD, t0 * 2048, [[W, 16], [16, nb], [1, K], [16 * W, 8]])
    nc.sync.dma_start(out=idxt[0:16, :], in_=src_w)
    nc.sync.dma_start(out=idxt[16:32, :], in_=idxt[0:16, :])
    nc.sync.dma_start(out=idxt[32:64, :], in_=idxt[0:32, :])
    nc.sync.dma_start(out=idxt[64:128, :], in_=idxt[0:64, :])

    PG = gp.tile([128, nb * 16, 64], F32, tag="pg")
    for bl in range(nb):
        for qh in range(2):
            nc.gpsimd.dma_gather(
                out_ap=PG[:, bl * 16 + qh * 8:bl * 16 + qh * 8 + 8, :],
                in_ap=PT_D.ap(),
                idxs_ap=idxt[:, bl * 128 + qh * 64:bl * 128 + qh * 64 + 64],
                num_idxs=1024, num_idxs_reg=1024, elem_size=64,
                single_packet=False,
            )

    AC = kp.tile([128, nb, 8, 128], BF16, tag="ac")
    B2C = kp.tile([128, nb, 8, 128], BF16, tag="b2c")
    C3C = kp.tile([128, nb, 8, 128], BF16, tag="c3c")

    for bl in range(nb):
        g = t0 + bl
        tb = TSTK[:, g * 128:(g + 1) * 128].unsqueeze(1).broadcast_to([128, 4, 128])
        rb = RSTK[:, g * 128:(g + 1) * 128].unsqueeze(1).broadcast_to([128, 4, 128])
        for q in range(2):
            # transposes: 4 kp blocks -> psum (128, 512)
            psa = pps2.tile([128, 512], F32, tag="a")
            for kk in range(4):
                kpi = q * 4 + kk
                blk = PG[:, bl * 16 + 2 * kpi:bl * 16 + 2 * kpi + 2, :]
                nc.tensor.transpose(psa[:, kk * 128:(kk + 1) * 128],
                                    blk, EYEt[:, :])
            nc.vector._custom_dve(
                RELU_ADD_ANT,
                out=AC[:, bl, 4 * q:4 * q + 4, :],
                in0=psa[:, :].rearrange("p (a b) -> p a b", a=4),
                in1=tb,
            )

            # conv2 (+R folded in as an identity-matmul accumulate)
            ps2t = pps1.tile([128, 512], F32, tag="u2")
            nc.tensor.matmul(ps2t[:, :], w2b[:, :],
                             AC[:, bl, 4 * q:4 * q + 4, :],
                             start=True, stop=False,
                             skip_group_check=True)
            nc.tensor.matmul(ps2t[:, :].rearrange("p (a b) -> p a b", a=4),
                             EYE16t[:, :], rb,
                             start=False, stop=True,
                             skip_group_check=True)
            nc.scalar.activation(B2C[:, bl, 4 * q:4 * q + 4, :], ps2t[:, :], RELU)

            # conv3 (accumulate two matmuls)
            ps3t = pps1.tile([128, 512], F32, tag="u2")
            nc.tensor.matmul(ps3t[:, :], w3a[:, :],
                             AC[:, bl, 4 * q:4 * q + 4, :],
                             start=True, stop=False)
            nc.tensor.matmul(ps3t[:, :], w3c[:, :],
                             B2C[:, bl, 4 * q:4 * q + 4, :],
                             start=False, stop=True)
            nc.scalar.activation(C3C[:, bl, 4 * q:4 * q + 4, :], ps3t[:, :], COPY)

    # maxes over k (kp in free dim, then k-parity across the 64-partition
    # halves), then DMA the chunk's output columns. Tree runs in bf16
    # (2x DVE); the last op per source emits f32.
    for (src, row0, add_s) in ((AC, 0, False), (B2C, 2 * G, False), (C3C, 3 * G, True)):
        m1 = kp.tile([128, nb, 4, 128], BF16, tag="m1")
        nc.vector.tensor_tensor(out=m1[:, :, :, :], in0=src[:, :, 0:4, :],
                                in1=src[:, :, 4:8, :], op=MAX)
        m2 = kp.tile([128, nb, 2, 128], BF16, tag="m2")
        nc.vector.tensor_tensor(out=m2[:, :, :, :], in0=m1[:, :, 0:2, :],
                                in1=m1[:, :, 2:4, :], op=MAX)
        red = kp.tile([128, nb, 128], BF16, tag=f"red{row0}")
        nc.vector.tensor_tensor(out=red[:, :, :], in0=m2[:, :, 0, :],
                                in1=m2[:, :, 1, :], op=MAX)
        hi = kp.tile([64, nb * 128], BF16, tag=f"hi{row0}")
        nc.scalar.activation(hi[:, :],
                             red[64:128, :, :].rearrange("p a n -> p (a n)"),
                             COPY)
        om = kp.tile([64, nb * 128], F32, tag=f"om{row0}")
        if add_s:
            omh = kp.tile([64, nb * 128], BF16, tag="omh")
            nc.vector.tensor_tensor(out=omh[:, :],
                                    in0=red[0:64, :, :].rearrange("p a n -> p (a n)"),
                                    in1=hi[:, :], op=MAX)
            nc.vector.tensor_tensor(out=om[:, :], in0=omh[:, :],
                                    in1=SCt[:, t0 * 128:(t0 + nb) * 128],
                                    op=ADD)
        else:
            nc.vector.tensor_tensor(out=om[:, :],
                                    in0=red[0:64, :, :].rearrange("p a n -> p (a n)"),
                                    in1=hi[:, :], op=MAX)
        nc.sync.dma_start(out=Y[row0:row0 + 64, t0 * 128:(t0 + nb) * 128],
                          in_=om[:, :])


def _prep_weights(W1, b1, W2, b2, W3, b3):
    W1 = np.asarray(W1, np.float32); W2 = np.asarray(W2, np.float32)
    W3 = np.asarray(W3, np.float32)
    b1 = np.asarray(b1, np.float32); b2 = np.asarray(b2, np.float32)
    b3 = np.asarray(b3, np.float32)
    W1a, W1b = W1[:, :64], W1[:, 64:]
    W2a, W2b = W2[:, :64], W2[:, 64:]
    W3a, W3b, W3c = W3[:, :64], W3[:, 64:128], W3[:, 128:]

    def blk(w):
        z = np.zeros((128, 128), np.float32)
        z[0:64, 0:64] = w.T
        z[64:128, 64:128] = w.T
        return z.astype(ml_dtypes.bfloat16)

    return {
        "NEGH": np.full((64, N), -0.5, np.float32),
        "ONESR": np.ones((1, N), np.float32),
        "WLTP": np.ascontiguousarray(W1a.T),
        "WLT": np.ascontiguousarray(np.vstack([(W1b - W1a).T, b1[None, :]])),
        "WLR": np.ascontiguousarray(np.vstack([W2b.T, b2[None, :]])),
        "WLS": np.ascontiguousarray(np.vstack([W3b.T, b3[None, :]])),
        "W2BLK": blk(W2a),
        "W3ABLK": blk(W3a),
        "W3CBLK": blk(W3c),
        "EYE": np.eye(128, dtype=np.float32),
        "EYE16": np.eye(128, dtype=np.float32).astype(ml_dtypes.bfloat16),
    }


_NC = None


def kernel(x, W1, b1, W2, b2, W3, b3):
    global _NC
    if _NC is None:
        _NC = build()
    x = np.asarray(x, np.float32)
    w = _prep_weights(W1, b1, W2, b2, W3, b3)
    in_maps = [{"x": np.ascontiguousarray(x[b]), "XR": np.ascontiguousarray(x[b]), **w}
               for b in range(B)]
    res = run_bass_kernel_spmd(_NC, in_maps, core_ids=list(range(B)))
    return np.stack([res.results[b]["y"] for b in range(B)], axis=0)
